# revision 2
# baseline (speedup 1.0000x reference)
"""TRN2 Bass kernel for block-sparse attention (nn_BlockSparseAttention).

kernel(**inputs) takes the FULL unsharded inputs (x [4,4096,1024], Wq/Wk/Wv/Wo
[1024,1024], bq/bk/bv/bo [1024]) and returns the full output [4,4096,1024].

Sharding: 8 cores = 4 batches x 2 head-halves (8 heads each). Each core
computes QKV projections, block-sparse attention, and a partial
out-projection [4096,1024]; the host sums the two half-partials plus bo.

On-chip formulation (keys-on-partitions, transpose-free):
  - host feeds xT = x[b].T so projections need no on-chip transpose;
    q^T/k^T come out [dout, tok] (weights stationary), v comes out
    [tok, dout] (activations stationary)
  - v carries one extra all-ones column per head (ones col = 0-weights +
    bias 1.0), so each PV matmul also emits the softmax denominator in
    PSUM row 64
  - scores are computed transposed, S^T = matmul(lhsT=k^T, rhs=q^T), with
    keys on partitions; no max-subtraction is needed (logits are O(1):
    x~N(0,1), W~N(0,1/1024) => S~N(0,1); exp cannot overflow fp32)
  - block 0 attends globally: a separate PSUM accumulation chain sums its
    PV over all key pieces
  - the softmax division happens at the end: denominators are gathered in
    32-aligned partition rows, reciprocal'd once, broadcast down 64
    partitions with a one-hot matmul, and multiplied into A^T
  - projections/out-projection run in float32r (full PE rate at N>=256,
    ~1e-4 rel err); attention operands (q^T/k^T/v/exp(S)) are bf16
    (full PE rate at any N)
"""
import os

import numpy as np

import concourse.bass as bass
import concourse.tile as tile
from concourse import mybir

F32 = mybir.dt.float32
F32R = mybir.dt.float32r
BF16 = mybir.dt.bfloat16
AF = mybir.ActivationFunctionType
SCALE = 1.0 / 8.0  # 1/sqrt(Dh=64)

N_CORES = 8
LAST_EXEC_NS = None


def _split_sync_waits(nc, cap=1):
    """This walrus build rejects >cap sync waits on one instruction; move
    excess waits onto same-engine no-ops placed just before (waits only
    become stricter in order, so this is semantics-preserving)."""
    for fn in nc.m.functions:
        for bb in fn.blocks:
            out = []
            for inst in bb.instructions:
                si = inst.sync_info
                waits = list(si.on_wait) if si and si.on_wait else []
                if len(waits) > cap:
                    extra, keep = waits[:-cap], waits[-cap:]
                    for i in range(0, len(extra), cap):
                        nop = mybir.InstNoOp(
                            name=nc.get_next_instruction_name(),
                            engine=inst.engine,
                            ins=[],
                            outs=[],
                            sync_info=mybir.SyncInfo(
                                on_wait=extra[i : i + cap], on_update=[]
                            ),
                        )
                        nc.register_instruction(nop)
                        out.append(nop)
                    si.on_wait = keep
                out.append(inst)
            bb.instructions[:] = out


def build_kernel(NT=4096, DM=1024, HL=8, DMO=1024):
    """One-core program; SPMD across 8 cores with different input slices."""
    DO = HL * 64          # local head dims
    DOV = HL * 65         # v with interleaved ones columns
    KC = DM // 128        # d_model chunks
    NJ = DO // 128        # head pairs (dout tiles)
    STOK = 512
    NS = NT // STOK
    NW2 = NT // 256       # 256-wide q windows
    assert DO % 128 == 0 and NT % 512 == 0

    nc = bass.Bass()
    xt_d = nc.dram_tensor("xt", [DM, NT], F32R, kind="ExternalInput")
    wq_d = nc.dram_tensor("wq", [DM, DO], F32R, kind="ExternalInput")
    wk_d = nc.dram_tensor("wk", [DM, DO], F32R, kind="ExternalInput")
    wv_d = nc.dram_tensor("wvp", [DM, DOV], F32R, kind="ExternalInput")
    wo_d = nc.dram_tensor("wo", [DO, DMO], F32R, kind="ExternalInput")
    bq_d = nc.dram_tensor("bq", [128, NJ], F32, kind="ExternalInput")
    bk_d = nc.dram_tensor("bk", [128, NJ], F32, kind="ExternalInput")
    bvb_d = nc.dram_tensor("bvb", [128, DOV], F32, kind="ExternalInput")
    sel_d = nc.dram_tensor("sel", [128, HL * 64 // 2], F32R, kind="ExternalInput")
    y_d = nc.dram_tensor("y", [NT, DMO], F32, kind="ExternalOutput")

    # v-projection output column split (psum bank holds 512 f32)
    if DOV <= 512:
        vchunks = [(0, DOV)]
    else:
        half = (DOV // 2 + 3) & ~3
        vchunks = [(0, half), (half, DOV - half)]

    with tile.TileContext(nc) as tc, nc.allow_low_precision(
        reason="attention operands intentionally bf16; matmul accum stays fp32"
    ):
        from contextlib import ExitStack

        with ExitStack() as ctx:
            dram = ctx.enter_context(tc.tile_pool(name="dram", bufs=1, space="DRAM"))
            qtd = dram.tile([DO, NT], BF16, tag="qtd")
            vvd = dram.tile([NT, DOV], BF16, tag="vvd")

            pers = ctx.enter_context(tc.tile_pool(name="pers", bufs=1))
            d_sb = pers.tile([128, 2 * NT], F32R, tag="dsb")
            sel = pers.tile([128, NJ * 64], F32R, tag="sel")
            nc.vector.memset(d_sb[:].bitcast(F32), 1.0)
            nc.sync.dma_start(sel[:], sel_d[:])
            # k^T resident for all head pairs; j=0's permuted q^T resident:
            # phase 2 starts the moment phase 1 ends, no DRAM round-trip
            kts = [pers.tile([128, NT], BF16, tag=f"kt{j}", name=f"kt{j}")
                   for j in range(NJ)]
            qt0 = pers.tile([128, NW2 * 320], BF16, tag="qt0")
            qt0w = qt0.rearrange("p (w c) -> p w c", c=320)

            # ---------------- phase 1: projections ----------------
            with (
                tc.tile_pool(name="p1w", bufs=1) as p1w,
                tc.tile_pool(name="p1x", bufs=2) as p1x,
                tc.tile_pool(name="p1st", bufs=4) as p1st,
                tc.tile_pool(name="p1ps", bufs=3, space="PSUM") as p1ps,
                tc.tile_pool(name="p1psv", bufs=4, space="PSUM") as p1psv,
            ):
                wqs = p1w.tile([128, KC * DO], F32R, tag="wqs")
                wks = p1w.tile([128, KC * DO], F32R, tag="wks")
                wvs = p1w.tile([128, KC * DOV], F32R, tag="wvs")
                bqs = p1w.tile([128, NJ], F32, tag="bqs")
                bks = p1w.tile([128, NJ], F32, tag="bks")
                bvbs = p1w.tile([128, DOV], F32, tag="bvbs")
                for c in range(KC):
                    r = slice(c * 128, (c + 1) * 128)
                    nc.sync.dma_start(wqs[:, c * DO : (c + 1) * DO], wq_d[r, :])
                    nc.sync.dma_start(wks[:, c * DO : (c + 1) * DO], wk_d[r, :])
                    nc.sync.dma_start(wvs[:, c * DOV : (c + 1) * DOV], wv_d[r, :])
                nc.sync.dma_start(bqs[:], bq_d[:])
                nc.sync.dma_start(bks[:], bk_d[:])
                nc.sync.dma_start(bvbs[:], bvb_d[:])

                for s in range(NS):
                    ts = slice(s * STOK, (s + 1) * STOK)
                    xts = p1x.tile([128, KC * STOK], F32R, tag="xts")
                    for c in range(KC):
                        nc.sync.dma_start(
                            xts[:, c * STOK : (c + 1) * STOK],
                            xt_d[c * 128 : (c + 1) * 128, ts],
                        )
                    for (wsb, bsb, isq) in ((wqs, bqs, True), (wks, bks, False)):
                        for j in range(NJ):
                            ps = p1ps.tile([128, STOK], F32, tag="psqk", name="psqk")
                            for c in range(KC):
                                nc.tensor.matmul(
                                    ps[:],
                                    wsb[:, c * DO + j * 128 : c * DO + (j + 1) * 128],
                                    xts[:, c * STOK : (c + 1) * STOK],
                                    start=(c == 0),
                                    stop=(c == KC - 1),
                                )
                            if not isq:
                                nc.scalar.activation(
                                    kts[j][:, ts], ps[:], AF.Identity,
                                    bias=bsb[:, j : j + 1],
                                )
                            elif j == 0:
                                nw_s = STOK // 256
                                nc.scalar.activation(
                                    qt0w[:, s * nw_s : (s + 1) * nw_s, 0:256],
                                    ps[:].rearrange("p (w c) -> p w c", c=256),
                                    AF.Identity, bias=bsb[:, j : j + 1],
                                )
                            else:
                                st = p1st.tile([128, STOK], BF16, tag="stq",
                                               name="stq")
                                nc.scalar.activation(
                                    st[:], ps[:], AF.Identity, bias=bsb[:, j : j + 1]
                                )
                                nc.sync.dma_start(
                                    qtd[j * 128 : (j + 1) * 128, ts], st[:]
                                )
                    for t in range(STOK // 128):
                        vst = p1st.tile([128, DOV], BF16, tag="stv")
                        for (o, wd) in vchunks:
                            ps = p1psv.tile([128, vchunks[0][1]], F32, tag="psv")
                            for c in range(KC):
                                nc.tensor.matmul(
                                    ps[:, 0:wd],
                                    xts[:, c * STOK + t * 128 : c * STOK + (t + 1) * 128],
                                    wvs[:, c * DOV + o : c * DOV + o + wd],
                                    start=(c == 0),
                                    stop=(c == KC - 1),
                                )
                            nc.vector.tensor_add(
                                vst[:, o : o + wd], bvbs[:, o : o + wd], ps[:, 0:wd]
                            )
                        nc.sync.dma_start(
                            vvd[s * STOK + t * 128 : s * STOK + (t + 1) * 128, :],
                            vst[:],
                        )
                # replicate j0's q0 block into every window's tail slot
                nc.gpsimd.dma_start(
                    qt0w[:, :, 256:320],
                    qt0[:, 0:64].rearrange("p (o c) -> p o c", o=1).to_broadcast(
                        [128, NW2, 64]
                    ),
                )
                # HAM warm-keeper: dependency-free matmuls bridge the
                # phase-1 -> phase-2 PE idle gap so the clock gate never
                # re-throttles (a PE-idle stretch halves the PE clock)
                fw = min(512, KC * DO)
                fill = p1ps.tile([128, 512], F32, tag="psqk", name="fill")
                for _ in range(40):
                    nc.tensor.matmul(
                        fill[:, 0:fw], wqs[:, 0:128], wqs[:, 0:fw],
                        start=True, stop=True,
                    )

            # ---------------- phases 2+3 ----------------
            atp = ctx.enter_context(tc.tile_pool(name="atp", bufs=1))
            ats = [atp.tile([128, NT], F32R, tag=f"at{j}", name=f"at{j}")
                   for j in range(NJ)]
            with (
                tc.tile_pool(name="p2qt", bufs=2) as p2qt,
                tc.tile_pool(name="p2vv", bufs=4) as p2vv,
                tc.tile_pool(name="p2e", bufs=1) as p2e,
                tc.tile_pool(name="p2o0", bufs=1) as p2o0,
                tc.tile_pool(name="p2ps", bufs=6, space="PSUM") as p2ps,
                tc.tile_pool(name="p2pt", bufs=2, space="PSUM") as p2pt,
            ):
                for j in range(NJ):
                    ktj = kts[j]
                    atj = ats[j]
                    if j == 0:
                        qt = qt0
                    else:
                        # q^T permuted: 256 window cols + the 64 q0 cols
                        qt = p2qt.tile([128, NW2 * 320], BF16, tag="qt")
                        qt3 = qt.rearrange("p (w c) -> p w c", c=320)
                        qsrc = qtd[j * 128 : (j + 1) * 128, :]
                        nc.sync.dma_start(
                            qt3[:, :, 0:256],
                            qsrc.rearrange("p (w c) -> p w c", c=256),
                        )
                        nc.sync.dma_start(
                            qt3[:, :, 256:320],
                            qsrc[:, 0:64].rearrange("p (o c) -> p o c", o=1)
                            .to_broadcast([128, NW2, 64]),
                        )
                    # block-0 PV accumulators (SBUF; fed from T-window tails)
                    o0 = [p2o0.tile([65, 64], F32, tag=f"o0_{hh}", name=f"o0_{hh}")
                          for hh in (0, 1)]
                    # E rings per piece type (A/B/C) per head
                    ering = [[p2e.tile([128, 3 * 320], BF16, tag=f"e{p}_{hh}",
                                       name=f"e{p}_{hh}") for hh in (0, 1)]
                             for p in range(3)]
                    cj = 2 * j * 65
                    vv_prev_c = None

                    def emit_pv(st):
                        w, sl, pieces = st
                        for hh in (0, 1):
                            vh = slice(hh * 65, hh * 65 + 65)
                            pt = p2pt.tile([65, 320], F32, tag="psT", name="psT")
                            for (p, klo, kn, vv, q0) in pieces:
                                et = ering[p][hh]
                                ncol = 320 if q0 else 256
                                nc.tensor.matmul(
                                    pt[0:65, 0:ncol],
                                    vv[0:kn, vh],
                                    et[0:kn, sl : sl + ncol],
                                    start=(p == 0), stop=(p == 2),
                                )
                            # block-0 accumulation (SBUF)
                            if w == 0:
                                nc.vector.tensor_copy(o0[hh][:], pt[0:65, 256:320])
                            else:
                                nc.vector.tensor_add(
                                    o0[hh][:], o0[hh][:], pt[0:65, 256:320]
                                )
                            # evacuate T_w
                            lo = 64 if w == 0 else 0
                            q_lo = 256 * w + lo
                            nW = 256 - lo
                            hr = slice(hh * 64, hh * 64 + 64)
                            dst = atj[hr, q_lo : q_lo + nW]
                            srcp = pt[0:64, lo : lo + nW]
                            if hh == 0:
                                nc.scalar.copy(dst, srcp)
                            else:
                                nc.vector.tensor_copy(dst, srcp)
                            nc.vector.tensor_copy(
                                d_sb[32 * j : 32 * j + 1,
                                     hh * NT + q_lo : hh * NT + q_lo + nW],
                                pt[64:65, lo : lo + nW],
                            )

                    pv_pending = None
                    for w in range(NW2):
                        sl = (w % 3) * 320
                        dmae = nc.gpsimd if (j == 0 and w < 3) else nc.sync
                        # key pieces: A (shared with prev C), B, C
                        if w == 0:
                            vva = p2vv.tile([64, 130], BF16, tag="vva0")
                            dmae.dma_start(vva[:], vvd[0:64, cj : cj + 130])
                            ka_lo, ka_n = 0, 64
                        else:
                            vva = vv_prev_c
                            ka_lo, ka_n = 256 * w - 64, 128
                        kb_lo = 256 * w + 64
                        vvb = p2vv.tile([128, 130], BF16, tag="vvb")
                        dmae.dma_start(vvb[:], vvd[kb_lo : kb_lo + 128, cj : cj + 130])
                        kc_lo = 256 * w + 192
                        kc_n = min(128, NT - kc_lo)
                        vvc = p2vv.tile([128, 130], BF16, tag="vvc")
                        dmae.dma_start(
                            vvc[0:kc_n, :], vvd[kc_lo : kc_lo + kc_n, cj : cj + 130]
                        )
                        # piece tuples: (type, key_lo, key_n, vv, has_q0)
                        pieces = [
                            (0, ka_lo, ka_n, vva, w == 0),
                            (1, kb_lo, 128, vvb, True),
                            (2, kc_lo, kc_n, vvc, True),
                        ]
                        # --- S^T: one matmul per (piece, head); heads
                        # interleaved so their h0/h64 row groups overlap ---
                        psS = {}
                        for (p, klo, kn, _vv, q0) in pieces:
                            ncol = 320 if q0 else 256
                            for hh in (0, 1):
                                hr = slice(hh * 64, hh * 64 + 64)
                                ps = p2ps.tile([128, 320], F32, tag="psS",
                                               name=f"psS{p}{hh}")
                                nc.tensor.matmul(
                                    ps[0:kn, 0:ncol],
                                    ktj[hr, klo : klo + kn],
                                    qt[hr, 320 * w : 320 * w + ncol],
                                    start=True, stop=True,
                                )
                                psS[(p, hh)] = ps
                        # --- PV of the PREVIOUS window (software pipeline) ---
                        if pv_pending is not None:
                            emit_pv(pv_pending)
                        # --- exp (full tile) + corner zeroing on GpSimd ---
                        for (p, klo, kn, _vv, q0) in pieces:
                            ncol = 320 if q0 else 256
                            for hh in (0, 1):
                                et = ering[p][hh]
                                nc.scalar.activation(
                                    et[0:kn, sl : sl + ncol],
                                    psS[(p, hh)][0:kn, 0:ncol],
                                    AF.Exp, scale=SCALE,
                                )
                                if p == 0 and w == 0:
                                    zs = [(0, 64, 0, 64), (0, 64, 128, 256)]
                                elif p == 0:
                                    zs = [(0, 64, 64, 256), (64, 128, 128, 256)]
                                elif p == 1 and w == 0:
                                    zs = [(0, 64, 0, 64), (0, 64, 192, 256),
                                          (64, 128, 0, 64)]
                                elif p == 1:
                                    zs = [(0, 64, 192, 256), (64, 128, 0, 64)]
                                else:
                                    zs = [(0, 64, 0, 128), (64, 128, 0, 192)]
                                for (r0, r1, c0, c1) in zs:
                                    if r0 >= kn:
                                        continue
                                    nc.gpsimd.memset(
                                        et[r0 : min(r1, kn), sl + c0 : sl + c1], 0.0
                                    )
                        pv_pending = (w, sl, pieces)
                        vv_prev_c = vvc
                    emit_pv(pv_pending)
                    # block-0 evacuation (from SBUF accumulators)
                    for hh in (0, 1):
                        bp = hh * 64
                        nc.vector.tensor_copy(atj[bp : bp + 64, 0:64], o0[hh][0:64, :])
                        nc.vector.tensor_copy(
                            d_sb[32 * j : 32 * j + 1, hh * NT : hh * NT + 64],
                            o0[hh][64:65, :],
                        )

            # ---------------- softmax division + out projection ----------------
            with (
                tc.tile_pool(name="p3w", bufs=1) as p3w,
                tc.tile_pool(name="p3st", bufs=4) as p3st,
                tc.tile_pool(name="pfill", bufs=1, space="PSUM") as pfill,
                tc.tile_pool(name="p2bc", bufs=1, space="PSUM") as p2bc,
                tc.tile_pool(name="p3ps", bufs=4, space="PSUM") as p3ps,
            ):
                wos = [p3w.tile([128, DMO], F32R, tag=f"wo{j}", name=f"wo{j}")
                       for j in range(NJ)]
                for j in range(NJ):
                    nc.sync.dma_start(wos[j][:], wo_d[j * 128 : (j + 1) * 128, :])
                # 1/d via exp(-ln d) on the scalar engine
                for hhalf in (0, 1):
                    dsl = slice(hhalf * NT, (hhalf + 1) * NT)
                    nc.scalar.activation(d_sb[:, dsl], d_sb[:, dsl], AF.Ln)
                    nc.scalar.activation(d_sb[:, dsl], d_sb[:, dsl], AF.Exp, scale=-1.0)
                # HAM warm-keeper across the division tail
                fill2 = pfill.tile([128, 512], F32, tag="fill2", name="fill2")
                sw = min(128, NJ * 64)
                fw2 = min(512, DMO)
                for _ in range(50):
                    nc.tensor.matmul(
                        fill2[0:sw, 0:fw2], sel[:, 0:sw], wos[0][:, 0:fw2],
                        start=True, stop=True,
                    )
                NNC = (DMO + 511) // 512
                for qc in range(NT // 512):
                    qs = slice(qc * 512, (qc + 1) * 512)
                    for j in range(NJ):
                        for hh in (0, 1):
                            bc = p2bc.tile([64, 512], F32, tag=f"bc{hh}", name=f"bc{hh}")
                            nc.tensor.matmul(
                                bc[:],
                                sel[:, j * 64 : (j + 1) * 64],
                                d_sb[:, hh * NT + qc * 512 : hh * NT + (qc + 1) * 512],
                                start=True, stop=True,
                            )
                            nc.vector.tensor_mul(
                                ats[j][hh * 64 : hh * 64 + 64, qs],
                                ats[j][hh * 64 : hh * 64 + 64, qs],
                                bc[:],
                            )
                    for t in range(4):
                        tg = qc * 4 + t
                        for n in range(NNC):
                            nw = min(512, DMO - n * 512)
                            ps = p3ps.tile([128, 512], F32, tag="psy")
                            for j in range(NJ):
                                nc.tensor.matmul(
                                    ps[:, 0:nw],
                                    ats[j][:, tg * 128 : (tg + 1) * 128],
                                    wos[j][:, n * 512 : n * 512 + nw],
                                    start=(j == 0),
                                    stop=(j == NJ - 1),
                                )
                            sty = p3st.tile([128, 512], F32, tag="sty")
                            nc.scalar.copy(sty[:, 0:nw], ps[:, 0:nw])
                            nc.sync.dma_start(
                                y_d[tg * 128 : (tg + 1) * 128, n * 512 : n * 512 + nw],
                                sty[:, 0:nw],
                            )
    _split_sync_waits(nc)
    return nc


# ---------------------------------------------------------------- host glue
def _make_sel(NJ):
    s = np.zeros((128, NJ * 64), np.float32)
    for j in range(NJ):
        s[32 * j, j * 64 : (j + 1) * 64] = 1.0
    return s


def shard_inputs(x, Wq, bq, Wk, bk, Wv, bv, Wo, bo):
    """Full inputs -> per-core in_maps. Core c: batch c//2, head-half c%2."""
    DM = Wq.shape[0]
    DO = Wq.shape[1] // 2
    HL = DO // 64
    DOV = HL * 65
    NJ = DO // 128
    in_maps = []
    cache = {}
    for core in range(N_CORES):
        b, g = core // 2, core % 2
        if g not in cache:
            sl = slice(g * DO, (g + 1) * DO)
            wvp = np.zeros((DM, DOV), np.float32)
            bvb_row = np.zeros((DOV,), np.float32)
            for h in range(HL):
                wvp[:, h * 65 : h * 65 + 64] = Wv[:, g * DO + h * 64 : g * DO + (h + 1) * 64]
                bvb_row[h * 65 : h * 65 + 64] = bv[g * DO + h * 64 : g * DO + (h + 1) * 64]
                bvb_row[h * 65 + 64] = 1.0
            cache[g] = dict(
                wq=np.ascontiguousarray(Wq[:, sl]),
                wk=np.ascontiguousarray(Wk[:, sl]),
                wvp=wvp,
                wo=np.ascontiguousarray(Wo[sl, :]),
                bq=np.ascontiguousarray(bq[sl].reshape(NJ, 128).T),
                bk=np.ascontiguousarray(bk[sl].reshape(NJ, 128).T),
                bvb=np.broadcast_to(bvb_row, (128, DOV)).copy(),
                sel=_make_sel(NJ),
            )
        m = dict(cache[g])
        m["xt"] = np.ascontiguousarray(x[b].T)
        in_maps.append(m)
    return in_maps


_NC_CACHE = {}


def kernel(x, Wq, bq, Wk, bk, Wv, bv, Wo, bo):
    global LAST_EXEC_NS
    x = np.asarray(x, dtype=np.float32)
    Wq, bq = np.asarray(Wq, np.float32), np.asarray(bq, np.float32)
    Wk, bk = np.asarray(Wk, np.float32), np.asarray(bk, np.float32)
    Wv, bv = np.asarray(Wv, np.float32), np.asarray(bv, np.float32)
    Wo, bo = np.asarray(Wo, np.float32), np.asarray(bo, np.float32)
    B, NT, DM = x.shape

    from concourse.bass_utils import run_bass_kernel_spmd

    key = (NT, DM)
    if key not in _NC_CACHE:
        _NC_CACHE[key] = build_kernel(NT=NT, DM=DM)
    nc = _NC_CACHE[key]

    in_maps = shard_inputs(x, Wq, bq, Wk, bk, Wv, bv, Wo, bo)
    trace = bool(int(os.environ.get("BSATTN_TRACE", "0")))
    res = run_bass_kernel_spmd(nc, in_maps, list(range(N_CORES)), trace=trace)
    LAST_EXEC_NS = res.exec_time_ns
    globals()["LAST_RESULT"] = res

    out = np.empty((B, NT, DM), np.float32)
    for b in range(B):
        out[b] = res.results[2 * b]["y"] + res.results[2 * b + 1]["y"] + bo
    return out

